# revision 8
# baseline (speedup 1.0000x reference)
import os
import sys
import numpy as np
from contextlib import ExitStack

os.environ.setdefault("JAX_COMPILATION_CACHE_DIR", "/tmp/jaxcache")

try:
    import concourse.bass as bass
except ImportError:
    sys.path.insert(0, "/opt/trn_rl_repo")
    import concourse.bass as bass
import concourse.bacc as bacc
import concourse.tile as tile
from concourse import mybir
from concourse.bass_utils import run_bass_kernel_spmd

FP = mybir.dt.float32
FR = mybir.dt.float32r
AL = mybir.AluOpType
AF = mybir.ActivationFunctionType

T = 500          # scan steps
B = 4096         # batch
D = 32           # state dim
H = 128          # hidden dim
NCORES = 8
BC = B // NCORES  # 512 batch per core (free dim)
W = 2             # waves per step
F = BC // W       # 256 free width per wave
DT = np.float32(0.01)
SQRT_DT = np.sqrt(DT).astype(np.float32)


def _build_nc(t_steps):
    nc = bacc.Bacc()
    y0p = nc.declare_dram_parameter("y0p", [D, BC], FR, isOutput=False)
    noisep = nc.declare_dram_parameter("noisep", [t_steps, D, BC], FP, isOutput=False)
    dw1t = nc.declare_dram_parameter("dw1t", [D, H], FR, isOutput=False)
    gw1t = nc.declare_dram_parameter("gw1t", [D, H], FR, isOutput=False)
    dw2t = nc.declare_dram_parameter("dw2t", [H, D], FR, isOutput=False)
    gw2t = nc.declare_dram_parameter("gw2t", [H, D], FR, isOutput=False)
    eyep = nc.declare_dram_parameter("eyep", [D, D], FR, isOutput=False)
    db1c = nc.declare_dram_parameter("db1c", [H, 1], FP, isOutput=False)
    gb1c = nc.declare_dram_parameter("gb1c", [H, 1], FP, isOutput=False)
    gb2c = nc.declare_dram_parameter("gb2c", [D, 1], FP, isOutput=False)
    dtdb2c = nc.declare_dram_parameter("dtdb2c", [D, 1], FP, isOutput=False)
    outp = nc.declare_dram_parameter("outp", [t_steps, D, BC], FP, isOutput=True)

    with ExitStack() as ctx:
        tc = ctx.enter_context(tile.TileContext(nc))
        wp = ctx.enter_context(tc.tile_pool(name="w", bufs=1))
        ypA = ctx.enter_context(tc.tile_pool(name="yA", bufs=3))
        ypB = ctx.enter_context(tc.tile_pool(name="yB", bufs=3))
        hp = ctx.enter_context(tc.tile_pool(name="h", bufs=1))
        nsp = ctx.enter_context(tc.tile_pool(name="ns", bufs=8))
        gp = ctx.enter_context(tc.tile_pool(name="gn", bufs=2))
        pp = ctx.enter_context(
            tc.tile_pool(name="ps", bufs=1, space=bass.MemorySpace.PSUM)
        )

        w_dw1 = wp.tile([D, H], FR)
        w_gw1 = wp.tile([D, H], FR)
        w_dw2 = wp.tile([H, D], FR)
        w_gw2 = wp.tile([H, D], FR)
        w_eye = wp.tile([D, D], FR)
        w_db1 = wp.tile([H, 1], FP)
        w_gb1 = wp.tile([H, 1], FP)
        w_gb2 = wp.tile([D, 1], FP)
        w_dtdb2 = wp.tile([D, 1], FP)
        for t_, d_ in [
            (w_dw1, dw1t), (w_gw1, gw1t), (w_dw2, dw2t), (w_gw2, gw2t),
            (w_eye, eyep), (w_db1, db1c), (w_gb1, gb1c), (w_gb2, gb2c),
            (w_dtdb2, dtdb2c),
        ]:
            nc.sync.dma_start(t_[:], d_[:])

        hd = hp.tile([H, BC], FR)
        hg = hp.tile([H, BC], FR)
        ps_hd = pp.tile([H, BC], FP)
        ps_hg = pp.tile([H, BC], FP)
        ps_d = pp.tile([D, BC], FP)
        ps_g = pp.tile([D, BC], FP)

        yw = [ypA.tile([D, F], FR, name="yA0"), ypB.tile([D, F], FR, name="yB0")]
        for w in range(W):
            nc.sync.dma_start(yw[w][:], y0p[:, w * F:(w + 1) * F])

        for k in range(t_steps):
            ns = nsp.tile([D, BC], FP)
            nc.gpsimd.dma_start(ns[:], noisep[k])

            y_new = [None, None]
            for w in range(W):
                s = slice(w * F, (w + 1) * F)
                y = yw[w]

                # PE: H_g, H_d = w1g@y, w1d@y ; psD opens with identity@y
                nc.tensor.matmul(ps_hg[:, s], w_gw1[:], y[:], start=True, stop=True)
                nc.tensor.matmul(ps_hd[:, s], w_dw1[:], y[:], start=True, stop=True)
                nc.tensor.matmul(ps_d[:, s], w_eye[:], y[:], start=True, stop=False)

                # relu on ACT for both nets (g first: it is on the critical cycle)
                nc.scalar.activation(hg[:, s], ps_hg[:, s], AF.Relu,
                                     bias=w_gb1[:], scale=1.0)
                nc.scalar.activation(hd[:, s], ps_hd[:, s], AF.Relu,
                                     bias=w_db1[:], scale=1.0)

                # MM2: G = w2g@hg ; psD += dt*w2d@hd  (now psD = y + dt*f - dt*db2)
                nc.tensor.matmul(ps_g[:, s], w_gw2[:], hg[:, s], start=True, stop=True)
                nc.tensor.matmul(ps_d[:, s], w_dw2[:], hd[:, s], start=False, stop=True)

                # gn = (G + gb2) * (sqrt_dt * noise)   [noise pre-scaled on host]
                gn = gp.tile([D, F], FP)
                nc.vector.scalar_tensor_tensor(
                    gn[:], ps_g[:, s], w_gb2[:], ns[:, s], AL.add, AL.mult
                )

                # y_new = (psD + dt*db2) + gn
                yn = (ypA if w == 0 else ypB).tile([D, F], FR, name=f"yn{w}")
                nc.vector.scalar_tensor_tensor(
                    yn[:], ps_d[:, s], w_dtdb2[:], gn[:], AL.add, AL.add
                )
                nc.sync.dma_start(outp[k][:, s], yn[:].bitcast(FP))
                y_new[w] = yn
            yw = y_new

    nc.finalize()
    return nc


_NC_CACHE = {}


def _get_nc(t_steps):
    if t_steps not in _NC_CACHE:
        _NC_CACHE[t_steps] = _build_nc(t_steps)
    return _NC_CACHE[t_steps]


def _r32(x):
    import ml_dtypes
    hi = x.astype(ml_dtypes.bfloat16).astype(np.float32)
    lo = (x - hi).astype(ml_dtypes.bfloat16).astype(np.float32)
    return hi + lo


def _run(inputs, trace=False, t_steps=T, tmpdir=None):
    y0 = np.asarray(inputs["y0"], np.float32)
    noise = np.asarray(inputs["noise"], np.float32)[:t_steps]
    dw1 = np.asarray(inputs["drift_w1"], np.float32)
    db1 = np.asarray(inputs["drift_b1"], np.float32)
    dw2 = np.asarray(inputs["drift_w2"], np.float32)
    db2 = np.asarray(inputs["drift_b2"], np.float32)
    gw1 = np.asarray(inputs["diff_w1"], np.float32)
    gb1 = np.asarray(inputs["diff_b1"], np.float32)
    gw2 = np.asarray(inputs["diff_w2"], np.float32)
    gb2 = np.asarray(inputs["diff_b2"], np.float32)

    shared = {
        "dw1t": _r32(np.ascontiguousarray(dw1.T)),
        "gw1t": _r32(np.ascontiguousarray(gw1.T)),
        "dw2t": _r32(np.ascontiguousarray(dw2.T * DT)),
        "gw2t": _r32(np.ascontiguousarray(gw2.T)),
        "eyep": np.eye(D, dtype=np.float32),
        "db1c": np.ascontiguousarray(db1[:, None]),
        "gb1c": np.ascontiguousarray(gb1[:, None]),
        "gb2c": np.ascontiguousarray(gb2[:, None]),
        "dtdb2c": np.ascontiguousarray((db2 * DT)[:, None]),
    }
    noise_s = noise * SQRT_DT

    in_maps = []
    for i in range(NCORES):
        sl = slice(i * BC, (i + 1) * BC)
        m = dict(shared)
        m["y0p"] = _r32(np.ascontiguousarray(y0[sl].T))
        m["noisep"] = np.ascontiguousarray(noise_s[:, sl, :].transpose(0, 2, 1))
        in_maps.append(m)

    nc = _get_nc(t_steps)
    try:
        res = run_bass_kernel_spmd(
            nc, in_maps, list(range(NCORES)), trace=trace, tmpdir=tmpdir
        )
    except ModuleNotFoundError:
        res = run_bass_kernel_spmd(nc, in_maps, list(range(NCORES)), trace=False)

    full = np.empty((t_steps + 1, B, D), np.float32)
    full[0] = y0
    for i in range(NCORES):
        full[1:, i * BC:(i + 1) * BC, :] = res.results[i]["outp"].transpose(0, 2, 1)
    return full, res.exec_time_ns


def kernel(**inputs):
    out, _ = _run(inputs, trace=False)
    return out
